# revision 25
# baseline (speedup 1.0000x reference)
"""DIN-style attention (MLP over [qt, k, qt-k, qt*k] + masked softmax) on 8 TRN2 cores.

Data-parallel over batch: each core handles 512 of 4096 rows.

Per-core device layout (feature-major, t-major tiles of all 512 local b's):
  - W1 is algebraically combined on-device: info@W1 = qp@(W1q+W1m) + k@(W1k-W1m) + (qp*k)@W1p,
    so the (qt-k) features never need materializing.
  - Layer 1 = three K=32 matmuls at distinct 32-row PE strips (run concurrently),
    accumulating into one PSUM bank [80, 512].
  - Sigmoids read PSUM directly with fused per-partition bias.
  - Layer 3 uses act2 chunks [40, 128] as the stationary so logits land as
    [128 b-partitions, t columns] in PSUM -- the exact layout softmax needs.
  - Softmax: copy_predicated mask -> max-reduce (negated) -> Exp with bias=-max and
    accumulated row sums -> reciprocal -> scale -> contiguous DMA out.
"""

import contextlib

import numpy as np

import concourse.bacc as bacc
import concourse.mybir as mybir
import concourse.tile as tile
from concourse.bass_utils import run_bass_kernel_spmd

N_CORES = 8
B, T, D = 4096, 200, 32
BC = B // N_CORES  # 512 rows per core
H1, H2 = 80, 40
NBLK = BC // 128   # 4 blocks of 128 b's
NEG_BIG = float(np.float32(-2.0**32 + 1.0))

F32 = mybir.dt.float32
F32R = mybir.dt.float32r
BF16 = mybir.dt.bfloat16
I8 = mybir.dt.int8
AF = mybir.ActivationFunctionType
ALU = mybir.AluOpType
AX = mybir.AxisListType

def _emit(nc, tc, es, d, TT, repeat=1):
    groups = TT // 4
    const = es.enter_context(tc.tile_pool(name="const", bufs=1))
    ktp = es.enter_context(tc.tile_pool(name="ktp", bufs=3))
    qkp = es.enter_context(tc.tile_pool(name="qkp", bufs=3))
    a1p = es.enter_context(tc.tile_pool(name="a1p", bufs=3))
    a2p = es.enter_context(tc.tile_pool(name="a2p", bufs=3))
    ps1p = es.enter_context(tc.tile_pool(name="ps1p", bufs=2, space="PSUM"))
    ps2p = es.enter_context(tc.tile_pool(name="ps2p", bufs=1, space="PSUM"))
    psLp = es.enter_context(tc.tile_pool(name="psLp", bufs=1, space="PSUM"))

    # ---- static tiles ----
    w1raw = const.tile([4 * D, H1], F32R)
    wrep = const.tile([128, 3 * H1], F32R)  # strip j: [W1q+W1m | W1k-W1m | W1p]
    wq = const.tile([D, D], F32R)
    w2 = const.tile([H1, H2], F32R)
    wfraw = const.tile([H2, 1], F32)
    wf = const.tile([H2, 1], BF16)
    b1s = const.tile([H1, 1], F32)
    b2s = const.tile([H2, 1], F32)
    bqs = const.tile([D, 1], F32)
    als = const.tile([D, 1], F32)
    qts = const.tile([D, BC], F32R)
    qp4 = const.tile([128, BC], F32R)  # qp^T replicated at 4 strips
    mki = const.tile([128, NBLK, TT], I8)
    negb = const.tile([128, 2 * TT], F32)
    tmpr = const.tile([D, BC], F32)
    tmpa = const.tile([D, BC], F32)
    tmpb = const.tile([D, BC], F32)
    mx = const.tile([128, NBLK], F32)
    sums = const.tile([128, NBLK], F32)
    rin = const.tile([128, NBLK], F32)
    expv = const.tile([128, NBLK, TT], F32)
    att = const.tile([128, NBLK, TT], F32)

    nc.sync.dma_start(out=w1raw, in_=d["W1"])
    nc.sync.dma_start(out=wq, in_=d["Wq"])
    nc.sync.dma_start(out=w2, in_=d["W2"])
    nc.sync.dma_start(out=wfraw, in_=d["Wf"])
    nc.sync.dma_start(out=b1s, in_=d["b1"])
    nc.sync.dma_start(out=b2s, in_=d["b2"])
    nc.vector.tensor_copy(wf, wfraw)
    nc.sync.dma_start(out=bqs, in_=d["bq"])
    nc.sync.dma_start(out=als, in_=d["alpha"])
    nc.sync.dma_start(out=qts, in_=d["qT"])
    nc.sync.dma_start(out=mki, in_=d["mki"])

    # combined W1 blocks, then replicate to strips 1..3.
    # HW verifier (NCC_IBIR297) requires equal base partitions for 2-input DVE
    # ops, so align the blocks to base 0 first; cross-partition moves go via DMA.
    t32 = const.tile([32, H1], F32R)
    t64 = const.tile([32, H1], F32R)
    nc.sync.dma_start(out=t32, in_=d["W1"][32:64, :])
    nc.sync.dma_start(out=t64, in_=d["W1"][64:96, :])
    nc.vector.tensor_add(wrep[0:32, 0:H1], w1raw[0:32, :], t64)
    nc.vector.tensor_sub(wrep[0:32, H1:2 * H1], t32, t64)
    nc.sync.dma_start(out=wrep[0:32, 2 * H1:3 * H1], in_=d["W1"][96:128, :])
    for j in range(1, 4):
        nc.sync.dma_start(out=wrep[32 * j:32 * j + 32, :], in_=wrep[0:32, :])

    # qp^T = prelu(Wq^T @ q^T + bq, alpha)
    ps0 = ps1p.tile([D, BC], F32, tag="ps1")
    nc.tensor.matmul(ps0, wq, qts, start=True, stop=True)
    nc.scalar.activation(tmpr, ps0, AF.Relu, bias=bqs)
    nc.vector.tensor_scalar(tmpa, ps0, bqs, 0.0, op0=ALU.add, op1=ALU.min)
    nc.vector.tensor_scalar(tmpb, tmpa, als, None, op0=ALU.mult)
    nc.vector.tensor_add(qp4[0:32, :], tmpr, tmpb)
    for j in range(1, 4):
        nc.sync.dma_start(out=qp4[32 * j:32 * j + 32, :], in_=qp4[0:32, :])

    nc.vector.memset(negb, NEG_BIG)

    logA = psLp.tile([128, 2 * TT], F32)
    logB = psLp.tile([128, 2 * TT], F32)

    for _rep in range(repeat):
        _main_pass(nc, d, TT, ktp, qkp, a1p, a2p, ps1p, ps2p, wrep, w2, wf, b1s,
                   b2s, qp4, mki, negb, mx, sums, rin, expv, att, logA, logB)


def _main_pass(nc, d, TT, ktp, qkp, a1p, a2p, ps1p, ps2p, wrep, w2, wf, b1s, b2s,
               qp4, mki, negb, mx, sums, rin, expv, att, logA, logB):
    # ---- main loop: groups of 4 slots (2 pairs) ----
    for g in range(TT // 4):
        kt = ktp.tile([128, BC], F32R)
        nc.sync.dma_start(
            out=kt,
            in_=d["kT"][4 * g:4 * g + 4].rearrange("tj f b -> (tj f) b"),
        )
        qk = qkp.tile([128, BC], F32R)
        nc.vector.tensor_mul(qk, qp4, kt)
        for pair in range(2):
            ps1 = ps1p.tile([H1, 2, BC], F32, tag="ps1")  # two banks, one per tile
            ps2 = ps2p.tile([H2, 2, BC], F32)             # two banks, one per tile
            for i in range(2):
                j = 2 * pair + i
                t = 4 * g + j
                s = slice(32 * j, 32 * j + 32)
                tp = (32 * j, 0)
                p1 = ps1[:, i, :]
                nc.tensor.matmul(p1, wrep[s, 0:H1], qp4[s, :], start=True,
                                 stop=False, tile_position=tp)
                nc.tensor.matmul(p1, wrep[s, H1:2 * H1], kt[s, :], start=False,
                                 stop=False, tile_position=tp)
                nc.tensor.matmul(p1, wrep[s, 2 * H1:3 * H1], qk[s, :], start=False,
                                 stop=True, tile_position=tp)
            a1 = a1p.tile([H1, 2, BC], F32R)
            nc.scalar.activation(a1, ps1, AF.Sigmoid, bias=b1s)
            for i in range(2):
                nc.tensor.matmul(ps2[:, i, :], w2, a1[:, i, :],
                                 start=True, stop=True)
            a2 = a2p.tile([H2, 2, BC], BF16)
            nc.scalar.activation(a2, ps2, AF.Sigmoid, bias=b2s)
            for i in range(2):
                j = 2 * pair + i
                t = 4 * g + j
                for jj in range(4):
                    lps = logA if jj < 2 else logB
                    col = (jj % 2) * TT + t
                    nc.tensor.matmul(
                        lps[:, col:col + 1],
                        a2[:, i, 128 * jj:128 * jj + 128],
                        wf,
                        start=True,
                        stop=True,
                    )

    # ---- masked softmax over t ----
    for lps, blk0 in ((logA, 0), (logB, 2)):
        lview = lps.rearrange("p (g t) -> p g t", g=2)
        nc.vector.copy_predicated(
            lview, mki[:, blk0:blk0 + 2, :], negb.rearrange("p (g t) -> p g t", g=2)
        )
        nc.vector.tensor_reduce(
            mx[:, blk0:blk0 + 2], lview, axis=AX.X, op=ALU.max, negate=True
        )
        for i in range(2):
            blk = blk0 + i
            nc.scalar.activation(
                expv[:, blk, :],
                lps[:, i * TT:(i + 1) * TT],
                AF.Exp,
                bias=mx[:, blk:blk + 1],
                accum_out=sums[:, blk:blk + 1],
            )
    nc.vector.reciprocal(rin, sums)
    for blk in range(NBLK):
        nc.vector.tensor_scalar(
            att[:, blk, :], expv[:, blk, :], rin[:, blk:blk + 1], None, op0=ALU.mult
        )
    nc.sync.dma_start(
        out=d["out"].rearrange("(blk p) t -> p blk t", blk=NBLK), in_=att
    )


def build(TT=T, repeat=1):
    nc = bacc.Bacc("TRN2", target_bir_lowering=False, debug=False,
                   num_devices=N_CORES)
    d = {
        "kT": nc.dram_tensor("kT", [TT, D, BC], F32R, kind="ExternalInput").ap(),
        "qT": nc.dram_tensor("qT", [D, BC], F32R, kind="ExternalInput").ap(),
        "mki": nc.dram_tensor("mki", [128, NBLK, TT], I8, kind="ExternalInput").ap(),
        "Wq": nc.dram_tensor("Wq", [D, D], F32R, kind="ExternalInput").ap(),
        "bq": nc.dram_tensor("bq", [D, 1], F32, kind="ExternalInput").ap(),
        "alpha": nc.dram_tensor("alpha", [D, 1], F32, kind="ExternalInput").ap(),
        "W1": nc.dram_tensor("W1", [4 * D, H1], F32R, kind="ExternalInput").ap(),
        "b1": nc.dram_tensor("b1", [H1, 1], F32, kind="ExternalInput").ap(),
        "W2": nc.dram_tensor("W2", [H1, H2], F32R, kind="ExternalInput").ap(),
        "b2": nc.dram_tensor("b2", [H2, 1], F32, kind="ExternalInput").ap(),
        "Wf": nc.dram_tensor("Wf", [H2, 1], F32, kind="ExternalInput").ap(),
        "out": nc.dram_tensor("out", [BC, TT], F32, kind="ExternalOutput").ap(),
    }
    with tile.TileContext(nc) as tc:
        with contextlib.ExitStack() as es:
            _emit(nc, tc, es, d, TT, repeat=repeat)
    nc.compile()
    return nc


def prepare(q, k, mask, Wq, bq, alpha, W1, b1, W2, b2, Wf):
    """Varlen packing: per batch row keep only its unmasked t's (plus padding to
    the global max count, rounded to a multiple of 4). Pure index manipulation.
    Returns (in_maps, TT, tidx)."""
    mask_np = np.asarray(mask)
    cnt = (mask_np != 0).sum(1)                      # unmasked count per row
    TT = int(-(-int(cnt.max()) // 4) * 4)            # round up to x4
    TT = max(TT, 8)
    order = np.argsort(mask_np == 0, axis=1, kind="stable")  # unmasked first
    tidx = np.ascontiguousarray(order[:, :TT])       # [B, TT]
    kc = np.take_along_axis(np.asarray(k), tidx[:, :, None], axis=1)  # [B, TT, D]
    pad = (np.arange(TT)[None, :] >= cnt[:, None])   # True on pad slots
    common = {
        "Wq": np.ascontiguousarray(Wq, np.float32),
        "bq": np.ascontiguousarray(bq, np.float32).reshape(D, 1),
        "alpha": np.ascontiguousarray(alpha, np.float32).reshape(D, 1),
        "W1": np.ascontiguousarray(W1, np.float32),
        "b1": np.ascontiguousarray(b1, np.float32).reshape(H1, 1),
        "W2": np.ascontiguousarray(W2, np.float32),
        "b2": np.ascontiguousarray(b2, np.float32).reshape(H2, 1),
        "Wf": np.ascontiguousarray(Wf, np.float32).reshape(H2, 1),
    }
    in_maps = []
    for c in range(N_CORES):
        sl = slice(c * BC, (c + 1) * BC)
        kcc = np.ascontiguousarray(kc[sl].transpose(1, 2, 0), np.float32)  # [TT, D, BC]
        qc = np.ascontiguousarray(np.asarray(q)[sl].T, np.float32)  # [D, BC]
        mc = pad[sl].astype(np.int8)
        mc = np.ascontiguousarray(mc.reshape(NBLK, 128, TT).transpose(1, 0, 2))
        m = dict(common)
        m.update({"kT": kcc, "qT": qc, "mki": mc})
        in_maps.append(m)
    return in_maps, TT, tidx


def postprocess(results, TT, tidx):
    attc = np.empty((B, TT), np.float32)
    for c in range(N_CORES):
        attc[c * BC:(c + 1) * BC] = results[c]["out"]
    out = np.zeros((B, T), np.float32)
    np.put_along_axis(out, tidx, attc, axis=1)
    return out.reshape(B, 1, T)


_NC_CACHE = {}


def kernel(**inputs):
    in_maps, TT, tidx = prepare(
        inputs["q"], inputs["k"], inputs["mask"], inputs["Wq"], inputs["bq"],
        inputs["alpha"], inputs["W1"], inputs["b1"], inputs["W2"], inputs["b2"],
        inputs["Wf"],
    )
    if TT not in _NC_CACHE:
        _NC_CACHE[TT] = build(TT=TT)
    nc = _NC_CACHE[TT]
    res = run_bass_kernel_spmd(nc, in_maps, core_ids=list(range(N_CORES)))
    return postprocess(res.results, TT, tidx)


# revision 26
# speedup vs baseline: 7.3612x; 7.3612x over previous
"""DIN-style attention (MLP over [qt, k, qt-k, qt*k] + masked softmax) on 8 TRN2 cores.

Data-parallel over batch: each core handles 512 of 4096 rows.

Per-core device layout (feature-major, t-major tiles of all 512 local b's):
  - W1 is algebraically combined on-device: info@W1 = qp@(W1q+W1m) + k@(W1k-W1m) + (qp*k)@W1p,
    so the (qt-k) features never need materializing.
  - Layer 1 = three K=32 matmuls at distinct 32-row PE strips (run concurrently),
    accumulating into one PSUM bank [80, 512].
  - Sigmoids read PSUM directly with fused per-partition bias.
  - Layer 3 uses act2 chunks [40, 128] as the stationary so logits land as
    [128 b-partitions, t columns] in PSUM -- the exact layout softmax needs.
  - Softmax: copy_predicated mask -> max-reduce (negated) -> Exp with bias=-max and
    accumulated row sums -> reciprocal -> scale -> contiguous DMA out.
"""

import contextlib

import numpy as np

import concourse.bacc as bacc
import concourse.mybir as mybir
import concourse.tile as tile
from concourse.bass_utils import run_bass_kernel_spmd

N_CORES = 8
B, T, D = 4096, 200, 32
BC = B // N_CORES  # 512 rows per core
H1, H2 = 80, 40
NBLK = BC // 128   # 4 blocks of 128 b's
NEG_BIG = float(np.float32(-2.0**32 + 1.0))

F32 = mybir.dt.float32
F32R = mybir.dt.float32r
BF16 = mybir.dt.bfloat16
I8 = mybir.dt.int8
AF = mybir.ActivationFunctionType
ALU = mybir.AluOpType
AX = mybir.AxisListType

def _emit(nc, tc, es, d, TT, repeat=1):
    groups = TT // 4
    const = es.enter_context(tc.tile_pool(name="const", bufs=1))
    ktp = es.enter_context(tc.tile_pool(name="ktp", bufs=4))
    qkp = es.enter_context(tc.tile_pool(name="qkp", bufs=4))
    a1p = es.enter_context(tc.tile_pool(name="a1p", bufs=3))
    a2p = es.enter_context(tc.tile_pool(name="a2p", bufs=3))
    ps1p = es.enter_context(tc.tile_pool(name="ps1p", bufs=2, space="PSUM"))
    ps2p = es.enter_context(tc.tile_pool(name="ps2p", bufs=1, space="PSUM"))
    psLp = es.enter_context(tc.tile_pool(name="psLp", bufs=1, space="PSUM"))

    # ---- static tiles ----
    w1raw = const.tile([4 * D, H1], F32R)
    wrep = const.tile([128, 3 * H1], F32R)  # strip j: [W1q+W1m | W1k-W1m | W1p]
    wq = const.tile([D, D], F32R)
    w2 = const.tile([H1, H2], F32R)
    wfraw = const.tile([H2, 1], F32)
    wf = const.tile([H2, 1], BF16)
    b1s = const.tile([H1, 1], F32)
    b2s = const.tile([H2, 1], F32)
    bqs = const.tile([D, 1], F32)
    als = const.tile([D, 1], F32)
    qts = const.tile([D, BC], F32R)
    qp4 = const.tile([128, BC], F32R)  # qp^T replicated at 4 strips
    mki = const.tile([128, NBLK, TT], I8)
    negb = const.tile([128, 2 * TT], F32)
    tmpr = const.tile([D, BC], F32)
    tmpa = const.tile([D, BC], F32)
    tmpb = const.tile([D, BC], F32)
    mx = const.tile([128, NBLK], F32)
    sums = const.tile([128, NBLK], F32)
    rin = const.tile([128, NBLK], F32)
    expv = const.tile([128, NBLK, TT], F32)
    att = const.tile([128, NBLK, TT], F32)

    nc.sync.dma_start(out=w1raw, in_=d["W1"])
    nc.sync.dma_start(out=wq, in_=d["Wq"])
    nc.sync.dma_start(out=w2, in_=d["W2"])
    nc.sync.dma_start(out=wfraw, in_=d["Wf"])
    nc.sync.dma_start(out=b1s, in_=d["b1"])
    nc.sync.dma_start(out=b2s, in_=d["b2"])
    nc.vector.tensor_copy(wf, wfraw)
    nc.sync.dma_start(out=bqs, in_=d["bq"])
    nc.sync.dma_start(out=als, in_=d["alpha"])
    nc.sync.dma_start(out=qts, in_=d["qT"])
    nc.sync.dma_start(out=mki, in_=d["mki"])

    # combined W1 blocks, then replicate to strips 1..3.
    # HW verifier (NCC_IBIR297) requires equal base partitions for 2-input DVE
    # ops, so align the blocks to base 0 first; cross-partition moves go via DMA.
    t32 = const.tile([32, H1], F32R)
    t64 = const.tile([32, H1], F32R)
    nc.sync.dma_start(out=t32, in_=d["W1"][32:64, :])
    nc.sync.dma_start(out=t64, in_=d["W1"][64:96, :])
    nc.vector.tensor_add(wrep[0:32, 0:H1], w1raw[0:32, :], t64)
    nc.vector.tensor_sub(wrep[0:32, H1:2 * H1], t32, t64)
    nc.sync.dma_start(out=wrep[0:32, 2 * H1:3 * H1], in_=d["W1"][96:128, :])
    for j in range(1, 4):
        nc.sync.dma_start(out=wrep[32 * j:32 * j + 32, :], in_=wrep[0:32, :])

    # qp^T = prelu(Wq^T @ q^T + bq, alpha)
    ps0 = ps1p.tile([D, BC], F32, tag="ps1")
    nc.tensor.matmul(ps0, wq, qts, start=True, stop=True)
    nc.vector.tensor_scalar(tmpr, ps0, bqs, 0.0, op0=ALU.add, op1=ALU.max)
    nc.vector.tensor_scalar(tmpa, ps0, bqs, 0.0, op0=ALU.add, op1=ALU.min)
    nc.vector.tensor_scalar(tmpb, tmpa, als, None, op0=ALU.mult)
    nc.vector.tensor_add(qp4[0:32, :], tmpr, tmpb)
    for j in range(1, 4):
        nc.sync.dma_start(out=qp4[32 * j:32 * j + 32, :], in_=qp4[0:32, :])

    nc.vector.memset(negb, NEG_BIG)

    logA = psLp.tile([128, 2 * TT], F32)
    logB = psLp.tile([128, 2 * TT], F32)

    for _rep in range(repeat):
        _main_pass(nc, d, TT, ktp, qkp, a1p, a2p, ps1p, ps2p, wrep, w2, wf, b1s,
                   b2s, qp4, mki, negb, mx, sums, rin, expv, att, logA, logB)


def _main_pass(nc, d, TT, ktp, qkp, a1p, a2p, ps1p, ps2p, wrep, w2, wf, b1s, b2s,
               qp4, mki, negb, mx, sums, rin, expv, att, logA, logB):
    # ---- main loop: groups of 4 slots (2 pairs) ----
    for g in range(TT // 4):
        kt = ktp.tile([128, BC], F32R)
        nc.sync.dma_start(
            out=kt,
            in_=d["kT"][4 * g:4 * g + 4].rearrange("tj f b -> (tj f) b"),
        )
        qk = qkp.tile([128, BC], F32R)
        nc.vector.tensor_mul(qk, qp4, kt)
        for pair in range(2):
            ps1 = ps1p.tile([H1, 2, BC], F32, tag="ps1")  # two banks, one per tile
            ps2 = ps2p.tile([H2, 2, BC], F32)             # two banks, one per tile
            for i in range(2):
                j = 2 * pair + i
                t = 4 * g + j
                s = slice(32 * j, 32 * j + 32)
                tp = (32 * j, 0)
                p1 = ps1[:, i, :]
                nc.tensor.matmul(p1, wrep[s, 0:H1], qp4[s, :], start=True,
                                 stop=False, tile_position=tp)
                nc.tensor.matmul(p1, wrep[s, H1:2 * H1], kt[s, :], start=False,
                                 stop=False, tile_position=tp)
                nc.tensor.matmul(p1, wrep[s, 2 * H1:3 * H1], qk[s, :], start=False,
                                 stop=True, tile_position=tp)
            a1 = a1p.tile([H1, 2, BC], F32R)
            nc.scalar.activation(a1, ps1, AF.Sigmoid, bias=b1s)
            for i in range(2):
                nc.tensor.matmul(ps2[:, i, :], w2, a1[:, i, :],
                                 start=True, stop=True)
            a2 = a2p.tile([H2, 2, BC], BF16)
            nc.scalar.activation(a2, ps2, AF.Sigmoid, bias=b2s)
            for i in range(2):
                j = 2 * pair + i
                t = 4 * g + j
                for jj in range(4):
                    lps = logA if jj < 2 else logB
                    col = (jj % 2) * TT + t
                    nc.tensor.matmul(
                        lps[:, col:col + 1],
                        a2[:, i, 128 * jj:128 * jj + 128],
                        wf,
                        start=True,
                        stop=True,
                    )

    # ---- masked softmax over t ----
    for lps, blk0 in ((logA, 0), (logB, 2)):
        lview = lps.rearrange("p (g t) -> p g t", g=2)
        nc.vector.copy_predicated(
            lview, mki[:, blk0:blk0 + 2, :], negb.rearrange("p (g t) -> p g t", g=2)
        )
        nc.vector.tensor_reduce(
            mx[:, blk0:blk0 + 2], lview, axis=AX.X, op=ALU.max, negate=True
        )
        for i in range(2):
            blk = blk0 + i
            nc.scalar.activation(
                expv[:, blk, :],
                lps[:, i * TT:(i + 1) * TT],
                AF.Exp,
                bias=mx[:, blk:blk + 1],
                accum_out=sums[:, blk:blk + 1],
            )
    nc.vector.reciprocal(rin, sums)
    for blk in range(NBLK):
        nc.vector.tensor_scalar(
            att[:, blk, :], expv[:, blk, :], rin[:, blk:blk + 1], None, op0=ALU.mult
        )
    nc.sync.dma_start(
        out=d["out"].rearrange("(blk p) t -> p blk t", blk=NBLK), in_=att
    )


def build(TT=T, repeat=1):
    nc = bacc.Bacc("TRN2", target_bir_lowering=False, debug=False,
                   num_devices=N_CORES)
    d = {
        "kT": nc.dram_tensor("kT", [TT, D, BC], F32R, kind="ExternalInput").ap(),
        "qT": nc.dram_tensor("qT", [D, BC], F32R, kind="ExternalInput").ap(),
        "mki": nc.dram_tensor("mki", [128, NBLK, TT], I8, kind="ExternalInput").ap(),
        "Wq": nc.dram_tensor("Wq", [D, D], F32R, kind="ExternalInput").ap(),
        "bq": nc.dram_tensor("bq", [D, 1], F32, kind="ExternalInput").ap(),
        "alpha": nc.dram_tensor("alpha", [D, 1], F32, kind="ExternalInput").ap(),
        "W1": nc.dram_tensor("W1", [4 * D, H1], F32R, kind="ExternalInput").ap(),
        "b1": nc.dram_tensor("b1", [H1, 1], F32, kind="ExternalInput").ap(),
        "W2": nc.dram_tensor("W2", [H1, H2], F32R, kind="ExternalInput").ap(),
        "b2": nc.dram_tensor("b2", [H2, 1], F32, kind="ExternalInput").ap(),
        "Wf": nc.dram_tensor("Wf", [H2, 1], F32, kind="ExternalInput").ap(),
        "out": nc.dram_tensor("out", [BC, TT], F32, kind="ExternalOutput").ap(),
    }
    with tile.TileContext(nc) as tc:
        with contextlib.ExitStack() as es:
            _emit(nc, tc, es, d, TT, repeat=repeat)
    nc.compile()
    return nc


def prepare(q, k, mask, Wq, bq, alpha, W1, b1, W2, b2, Wf):
    """Varlen packing: per batch row keep only its unmasked t's (plus padding to
    the global max count, rounded to a multiple of 4). Pure index manipulation.
    Returns (in_maps, TT, tidx)."""
    mask_np = np.asarray(mask)
    cnt = (mask_np != 0).sum(1)                      # unmasked count per row
    TT = int(-(-int(cnt.max()) // 4) * 4)            # round up to x4
    TT = max(TT, 8)
    order = np.argsort(mask_np == 0, axis=1, kind="stable")  # unmasked first
    tidx = np.ascontiguousarray(order[:, :TT])       # [B, TT]
    kc = np.take_along_axis(np.asarray(k), tidx[:, :, None], axis=1)  # [B, TT, D]
    pad = (np.arange(TT)[None, :] >= cnt[:, None])   # True on pad slots
    common = {
        "Wq": np.ascontiguousarray(Wq, np.float32),
        "bq": np.ascontiguousarray(bq, np.float32).reshape(D, 1),
        "alpha": np.ascontiguousarray(alpha, np.float32).reshape(D, 1),
        "W1": np.ascontiguousarray(W1, np.float32),
        "b1": np.ascontiguousarray(b1, np.float32).reshape(H1, 1),
        "W2": np.ascontiguousarray(W2, np.float32),
        "b2": np.ascontiguousarray(b2, np.float32).reshape(H2, 1),
        "Wf": np.ascontiguousarray(Wf, np.float32).reshape(H2, 1),
    }
    in_maps = []
    for c in range(N_CORES):
        sl = slice(c * BC, (c + 1) * BC)
        kcc = np.ascontiguousarray(kc[sl].transpose(1, 2, 0), np.float32)  # [TT, D, BC]
        qc = np.ascontiguousarray(np.asarray(q)[sl].T, np.float32)  # [D, BC]
        mc = pad[sl].astype(np.int8)
        mc = np.ascontiguousarray(mc.reshape(NBLK, 128, TT).transpose(1, 0, 2))
        m = dict(common)
        m.update({"kT": kcc, "qT": qc, "mki": mc})
        in_maps.append(m)
    return in_maps, TT, tidx


def postprocess(results, TT, tidx):
    attc = np.empty((B, TT), np.float32)
    for c in range(N_CORES):
        attc[c * BC:(c + 1) * BC] = results[c]["out"]
    out = np.zeros((B, T), np.float32)
    np.put_along_axis(out, tidx, attc, axis=1)
    return out.reshape(B, 1, T)


_NC_CACHE = {}


def kernel(**inputs):
    in_maps, TT, tidx = prepare(
        inputs["q"], inputs["k"], inputs["mask"], inputs["Wq"], inputs["bq"],
        inputs["alpha"], inputs["W1"], inputs["b1"], inputs["W2"], inputs["b2"],
        inputs["Wf"],
    )
    if TT not in _NC_CACHE:
        _NC_CACHE[TT] = build(TT=TT)
    nc = _NC_CACHE[TT]
    res = run_bass_kernel_spmd(nc, in_maps, core_ids=list(range(N_CORES)))
    return postprocess(res.results, TT, tidx)


# revision 27
# speedup vs baseline: 19.1412x; 2.6003x over previous
"""DIN-style attention (MLP over [qt, k, qt-k, qt*k] + masked softmax) on 8 TRN2 cores.

Data-parallel over batch: each core handles 512 of 4096 rows.

Per-core device layout (feature-major, t-major tiles of all 512 local b's):
  - W1 is algebraically combined on-device: info@W1 = qp@(W1q+W1m) + k@(W1k-W1m) + (qp*k)@W1p,
    so the (qt-k) features never need materializing.
  - Layer 1 = three K=32 matmuls at distinct 32-row PE strips (run concurrently),
    accumulating into one PSUM bank [80, 512].
  - Sigmoids read PSUM directly with fused per-partition bias.
  - Layer 3 uses act2 chunks [40, 128] as the stationary so logits land as
    [128 b-partitions, t columns] in PSUM -- the exact layout softmax needs.
  - Softmax: copy_predicated mask -> max-reduce (negated) -> Exp with bias=-max and
    accumulated row sums -> reciprocal -> scale -> contiguous DMA out.
"""

import contextlib

import numpy as np

import concourse.bacc as bacc
import concourse.mybir as mybir
import concourse.tile as tile
from concourse.bass_utils import run_bass_kernel_spmd

N_CORES = 8
B, T, D = 4096, 200, 32
BC = B // N_CORES  # 512 rows per core
H1, H2 = 80, 40
NBLK = BC // 128   # 4 blocks of 128 b's
NEG_BIG = float(np.float32(-2.0**32 + 1.0))

F32 = mybir.dt.float32
F32R = mybir.dt.float32r
BF16 = mybir.dt.bfloat16
I8 = mybir.dt.int8
AF = mybir.ActivationFunctionType
ALU = mybir.AluOpType
AX = mybir.AxisListType

def _emit(nc, tc, es, d, TT, repeat=1):
    groups = TT // 4
    const = es.enter_context(tc.tile_pool(name="const", bufs=1))
    ktp = es.enter_context(tc.tile_pool(name="ktp", bufs=4))
    qkp = es.enter_context(tc.tile_pool(name="qkp", bufs=4))
    a1p = es.enter_context(tc.tile_pool(name="a1p", bufs=3))
    a2p = es.enter_context(tc.tile_pool(name="a2p", bufs=3))
    ps1p = es.enter_context(tc.tile_pool(name="ps1p", bufs=2, space="PSUM"))
    ps2p = es.enter_context(tc.tile_pool(name="ps2p", bufs=1, space="PSUM"))
    psLp = es.enter_context(tc.tile_pool(name="psLp", bufs=1, space="PSUM"))

    # ---- static tiles ----
    w1raw = const.tile([4 * D, H1], F32R)
    wrep = const.tile([128, 3 * H1], F32R)  # strip j: [W1q+W1m | W1k-W1m | W1p]
    wq = const.tile([D, D], F32R)
    w2 = const.tile([H1, H2], F32R)
    wfraw = const.tile([H2, 1], F32)
    wf = const.tile([H2, 1], BF16)
    b1s = const.tile([H1, 1], F32)
    b2s = const.tile([H2, 1], F32)
    bqs = const.tile([D, 1], F32)
    als = const.tile([D, 1], F32)
    qts = const.tile([D, BC], F32R)
    qp4 = const.tile([128, BC], F32R)  # qp^T replicated at 4 strips
    mki = const.tile([128, NBLK, TT], I8)
    negb = const.tile([128, 2 * TT], F32)
    tmpr = const.tile([D, BC], F32)
    tmpa = const.tile([D, BC], F32)
    tmpb = const.tile([D, BC], F32)
    mx = const.tile([128, NBLK], F32)
    sums = const.tile([128, NBLK], F32)
    rin = const.tile([128, NBLK], F32)
    expv = const.tile([128, NBLK, TT], F32)
    att = const.tile([128, NBLK, TT], F32)

    nc.sync.dma_start(out=w1raw, in_=d["W1"])
    nc.sync.dma_start(out=wq, in_=d["Wq"])
    nc.sync.dma_start(out=w2, in_=d["W2"])
    nc.sync.dma_start(out=wfraw, in_=d["Wf"])
    nc.sync.dma_start(out=b1s, in_=d["b1"])
    nc.sync.dma_start(out=b2s, in_=d["b2"])
    nc.vector.tensor_copy(wf, wfraw)
    nc.sync.dma_start(out=bqs, in_=d["bq"])
    nc.sync.dma_start(out=als, in_=d["alpha"])
    nc.sync.dma_start(out=qts, in_=d["qT"])
    nc.sync.dma_start(out=mki, in_=d["mki"])

    # combined W1 blocks, then replicate to strips 1..3.
    # HW verifier (NCC_IBIR297) requires equal base partitions for 2-input DVE
    # ops, so align the blocks to base 0 first; cross-partition moves go via DMA.
    t32 = const.tile([32, H1], F32R)
    t64 = const.tile([32, H1], F32R)
    nc.sync.dma_start(out=t32, in_=d["W1"][32:64, :])
    nc.sync.dma_start(out=t64, in_=d["W1"][64:96, :])
    nc.vector.tensor_add(wrep[0:32, 0:H1], w1raw[0:32, :], t64)
    nc.vector.tensor_sub(wrep[0:32, H1:2 * H1], t32, t64)
    nc.sync.dma_start(out=wrep[0:32, 2 * H1:3 * H1], in_=d["W1"][96:128, :])
    for j in range(1, 4):
        nc.sync.dma_start(out=wrep[32 * j:32 * j + 32, :], in_=wrep[0:32, :])

    # qp^T = prelu(Wq^T @ q^T + bq, alpha)
    ps0 = ps1p.tile([D, BC], F32, tag="ps1")
    nc.tensor.matmul(ps0, wq, qts, start=True, stop=True)
    nc.vector.tensor_scalar(tmpr, ps0, bqs, 0.0, op0=ALU.add, op1=ALU.max)
    nc.vector.tensor_scalar(tmpa, ps0, bqs, 0.0, op0=ALU.add, op1=ALU.min)
    nc.vector.tensor_scalar(tmpb, tmpa, als, None, op0=ALU.mult)
    nc.vector.tensor_add(qp4[0:32, :], tmpr, tmpb)
    for j in range(1, 4):
        nc.sync.dma_start(out=qp4[32 * j:32 * j + 32, :], in_=qp4[0:32, :])

    nc.vector.memset(negb, NEG_BIG)

    logA = psLp.tile([128, 2 * TT], F32)
    logB = psLp.tile([128, 2 * TT], F32)

    for _rep in range(repeat):
        _main_pass(nc, d, TT, ktp, qkp, a1p, a2p, ps1p, ps2p, wrep, w2, wf, b1s,
                   b2s, qp4, mki, negb, mx, sums, rin, expv, att, logA, logB)


def _main_pass(nc, d, TT, ktp, qkp, a1p, a2p, ps1p, ps2p, wrep, w2, wf, b1s, b2s,
               qp4, mki, negb, mx, sums, rin, expv, att, logA, logB):
    # ---- main loop: groups of 4 slots (2 pairs) ----
    for g in range(TT // 4):
        kt = ktp.tile([128, BC], F32R)
        nc.sync.dma_start(
            out=kt,
            in_=d["kT"][4 * g:4 * g + 4].rearrange("tj f b -> (tj f) b"),
        )
        qk = qkp.tile([128, BC], F32R)
        nc.vector.tensor_mul(qk, qp4, kt)
        for pair in range(2):
            ps1 = ps1p.tile([H1, 2, BC], F32, tag="ps1")  # two banks, one per tile
            ps2 = ps2p.tile([H2, 2, BC], F32)             # two banks, one per tile
            for i in range(2):
                j = 2 * pair + i
                t = 4 * g + j
                s = slice(32 * j, 32 * j + 32)
                tp = (32 * j, 0)
                p1 = ps1[:, i, :]
                nc.tensor.matmul(p1, wrep[s, 0:H1], qp4[s, :], start=True,
                                 stop=False, tile_position=tp)
                nc.tensor.matmul(p1, wrep[s, H1:2 * H1], kt[s, :], start=False,
                                 stop=False, tile_position=tp)
                nc.tensor.matmul(p1, wrep[s, 2 * H1:3 * H1], qk[s, :], start=False,
                                 stop=True, tile_position=tp)
            a1 = a1p.tile([H1, 2, BC], F32R)
            nc.scalar.activation(a1, ps1, AF.Sigmoid, bias=b1s)
            for i in range(2):
                nc.tensor.matmul(ps2[:, i, :], w2, a1[:, i, :],
                                 start=True, stop=True)
            a2 = a2p.tile([H2, 2, BC], BF16)
            nc.scalar.activation(a2, ps2, AF.Sigmoid, bias=b2s)
            for i in range(2):
                j = 2 * pair + i
                t = 4 * g + j
                for jj in range(4):
                    lps = logA if jj < 2 else logB
                    col = (jj % 2) * TT + t
                    nc.tensor.matmul(
                        lps[:, col:col + 1],
                        a2[:, i, 128 * jj:128 * jj + 128],
                        wf,
                        start=True,
                        stop=True,
                    )

    # ---- masked softmax over t ----
    for lps, blk0 in ((logA, 0), (logB, 2)):
        lview = lps.rearrange("p (g t) -> p g t", g=2)
        nc.vector.copy_predicated(
            lview, mki[:, blk0:blk0 + 2, :], negb.rearrange("p (g t) -> p g t", g=2)
        )
        nc.vector.tensor_reduce(
            mx[:, blk0:blk0 + 2], lview, axis=AX.X, op=ALU.max, negate=True
        )
        for i in range(2):
            blk = blk0 + i
            nc.scalar.activation(
                expv[:, blk, :],
                lps[:, i * TT:(i + 1) * TT],
                AF.Exp,
                bias=mx[:, blk:blk + 1],
                accum_out=sums[:, blk:blk + 1],
            )
    nc.vector.reciprocal(rin, sums)
    for blk in range(NBLK):
        nc.vector.tensor_scalar(
            att[:, blk, :], expv[:, blk, :], rin[:, blk:blk + 1], None, op0=ALU.mult
        )
    nc.sync.dma_start(
        out=d["out"].rearrange("(blk p) t -> p blk t", blk=NBLK), in_=att
    )


def build(TT=T, repeat=1):
    nc = bacc.Bacc("TRN2", target_bir_lowering=False, debug=False,
                   num_devices=N_CORES)
    d = {
        "kT": nc.dram_tensor("kT", [TT, D, BC], F32R, kind="ExternalInput").ap(),
        "qT": nc.dram_tensor("qT", [D, BC], F32R, kind="ExternalInput").ap(),
        "mki": nc.dram_tensor("mki", [128, NBLK, TT], I8, kind="ExternalInput").ap(),
        "Wq": nc.dram_tensor("Wq", [D, D], F32R, kind="ExternalInput").ap(),
        "bq": nc.dram_tensor("bq", [D, 1], F32, kind="ExternalInput").ap(),
        "alpha": nc.dram_tensor("alpha", [D, 1], F32, kind="ExternalInput").ap(),
        "W1": nc.dram_tensor("W1", [4 * D, H1], F32R, kind="ExternalInput").ap(),
        "b1": nc.dram_tensor("b1", [H1, 1], F32, kind="ExternalInput").ap(),
        "W2": nc.dram_tensor("W2", [H1, H2], F32R, kind="ExternalInput").ap(),
        "b2": nc.dram_tensor("b2", [H2, 1], F32, kind="ExternalInput").ap(),
        "Wf": nc.dram_tensor("Wf", [H2, 1], F32, kind="ExternalInput").ap(),
        "out": nc.dram_tensor("out", [BC, TT], F32, kind="ExternalOutput").ap(),
    }
    with tile.TileContext(nc) as tc:
        with contextlib.ExitStack() as es:
            _emit(nc, tc, es, d, TT, repeat=repeat)
    nc.compile()
    return nc


def prepare(q, k, mask, Wq, bq, alpha, W1, b1, W2, b2, Wf):
    """Varlen packing: per batch row keep only its unmasked t's (plus padding to
    the global max count, rounded to a multiple of 4). Pure index manipulation.
    Returns (in_maps, TT, tidx)."""
    mask_np = np.asarray(mask)
    cnt = (mask_np != 0).sum(1)                      # unmasked count per row
    if cnt.min() == 0:
        # a fully-masked row needs the uniform-softmax semantics; identity
        # "compaction" reproduces the uncompacted kernel exactly
        TT = T
        tidx = np.ascontiguousarray(np.tile(np.arange(T), (B, 1)))
        pad = mask_np == 0
    else:
        TT = int(-(-int(cnt.max()) // 4) * 4)        # round up to x4
        TT = max(TT, 8)
        order = np.argsort(mask_np == 0, axis=1, kind="stable")  # unmasked first
        tidx = np.ascontiguousarray(order[:, :TT])   # [B, TT]
        pad = (np.arange(TT)[None, :] >= cnt[:, None])  # True on pad slots
    kc = np.take_along_axis(np.asarray(k), tidx[:, :, None], axis=1)  # [B, TT, D]
    common = {
        "Wq": np.ascontiguousarray(Wq, np.float32),
        "bq": np.ascontiguousarray(bq, np.float32).reshape(D, 1),
        "alpha": np.ascontiguousarray(alpha, np.float32).reshape(D, 1),
        "W1": np.ascontiguousarray(W1, np.float32),
        "b1": np.ascontiguousarray(b1, np.float32).reshape(H1, 1),
        "W2": np.ascontiguousarray(W2, np.float32),
        "b2": np.ascontiguousarray(b2, np.float32).reshape(H2, 1),
        "Wf": np.ascontiguousarray(Wf, np.float32).reshape(H2, 1),
    }
    in_maps = []
    for c in range(N_CORES):
        sl = slice(c * BC, (c + 1) * BC)
        kcc = np.ascontiguousarray(kc[sl].transpose(1, 2, 0), np.float32)  # [TT, D, BC]
        qc = np.ascontiguousarray(np.asarray(q)[sl].T, np.float32)  # [D, BC]
        mc = pad[sl].astype(np.int8)
        mc = np.ascontiguousarray(mc.reshape(NBLK, 128, TT).transpose(1, 0, 2))
        m = dict(common)
        m.update({"kT": kcc, "qT": qc, "mki": mc})
        in_maps.append(m)
    return in_maps, TT, tidx


def postprocess(results, TT, tidx):
    attc = np.empty((B, TT), np.float32)
    for c in range(N_CORES):
        attc[c * BC:(c + 1) * BC] = results[c]["out"]
    out = np.zeros((B, T), np.float32)
    np.put_along_axis(out, tidx, attc, axis=1)
    return out.reshape(B, 1, T)


_NC_CACHE = {}


def kernel(**inputs):
    in_maps, TT, tidx = prepare(
        inputs["q"], inputs["k"], inputs["mask"], inputs["Wq"], inputs["bq"],
        inputs["alpha"], inputs["W1"], inputs["b1"], inputs["W2"], inputs["b2"],
        inputs["Wf"],
    )
    if TT not in _NC_CACHE:
        _NC_CACHE[TT] = build(TT=TT)
    nc = _NC_CACHE[TT]
    res = run_bass_kernel_spmd(nc, in_maps, core_ids=list(range(N_CORES)))
    return postprocess(res.results, TT, tidx)
